# revision 1
# baseline (speedup 1.0000x reference)
"""Trainium2 Bass kernel for nn_CCL__69277822485245 (spectral conv via DCT/FFT).

Math: the reference's rFFT along W cancels into a circular 5-tap convolution,
and the DCT-II sandwich M @ diag(D[:,s]) @ D collapses into 5 dense 128x128
matrices G_s (precomputed on host). Per batch element:

    u_s[i, m, w] = sum_h G_s[m, h] x[i, h, w]                  (stage 1)
    out[o, m, n] = sum_{s,t,i} W[o,i,s,t] u_s[i, m, (n-t)%W] + bias[o]   (stage 2)

Sharding: data-parallel over batch B=8 across the 8 NeuronCores (1 each).

Layouts (per core):
  stage 1: per output column w, one matmul
      lhsT = xdup[h=128, di=128]    (x duplicated on the host so the output
                                     partition dim carries (d, i) pairs)
      rhs  = G^T[h=128, (s5, m64)]  (m in halves of 64 -> N=320; G s-order
                                     is [0,2,4,1,3] so each half's psum->u
                                     copy is a contiguous column slice)
      out  = psum[(d,i)=128, (sidx, m)]
      psum->SBUF casts split the halves: partitions 0-63 keep s={0,2,4}
      (slots 0..2), partitions 64-127 keep s={1,3} (slots 0..1), batched
      two w-columns per cast. SBUF u[(d,i), (slot, j, m)] -- j-major-of-m
      so stage-2 reads contiguous (j,m) runs.
  stage 2: for each t (same shift for both halves) and slot c:
      one K=128 matmul contracts (i, s=2c) on partitions 0-63 and
      (i, s=2c+1) on 64-127 simultaneously (c=2: K=64, s=4 only);
      15 sequential PSUM-accumulating passes, N = (j8, m64) = 512 contiguous.
      Bias added during the single per-block PSUM->SBUF evac (ScalarE).

DTYPE selects the matmul operand precision:
  "bf16": fastest (1 cyc/row + fast weight load), rel err ~ 3e-3
  "f32r": TF32-like (~2 cyc/row), rel err ~ 2e-4
  "f32" : exact fp32 (4 cyc/row), slowest
"""

import numpy as np

H = 128
W = 128
CI = 64
CO = 128
KH = 5
KW = 5
B = 8

MH = 64          # m-half processed per outer iteration
WB = 16          # w-block
HALO = 4         # extra back-columns for the t-shifts
WEXT = WB + HALO
NSLOT = 3        # s-slots per partition half (s = 2c + d)
JT = 8           # j-tile inside stage 2 (N = JT*MH = 512)

DTYPE = "bf16"

_PROG = None
_CONSTS = None
_RUN_OPTS = {}     # test harness may set e.g. {"trace": True, "trace_cores": [0]}
_LAST_RESULT = None


def _np_dt():
    if DTYPE == "bf16":
        import ml_dtypes
        return ml_dtypes.bfloat16
    return np.float32


def _build_consts():
    n = np.arange(H, dtype=np.float64)
    ang = np.pi * (2.0 * n[None, :] + 1.0) * n[:, None] / (2.0 * H)  # [k, h]
    D = 2.0 * np.cos(ang)
    wgt = np.where(n == 0, 0.5, 1.0)
    M = (np.cos(ang).T * wgt[None, :]) / (2.0 * H)                    # [m, k]
    G = np.stack([M @ (D[:, s:s + 1] * D) for s in range(KH)])        # [s, m, h]
    G = G[[0, 2, 4, 1, 3]]   # s-order so each half's psum->u copy is contiguous
    # rhs layout [h, (mh, sidx, ml)]: col = mh*320 + sidx*64 + ml
    GT = (G.transpose(2, 0, 1)                # [h, s, m]
            .reshape(H, KH, 2, MH)            # [h, s, mh, ml]
            .transpose(0, 2, 1, 3)            # [h, mh, s, ml]
            .reshape(H, KH * H))
    return np.ascontiguousarray(GT).astype(_np_dt())


def _build_program():
    import concourse.mybir as mybir
    import concourse.tile as tile
    from concourse import bacc

    f32 = mybir.dt.float32
    mmdt = {"bf16": mybir.dt.bfloat16,
            "f32r": mybir.dt.float32r,
            "f32": mybir.dt.float32}[DTYPE]

    nc = bacc.Bacc("TRN2", target_bir_lowering=False, debug=False,
                   enable_asserts=False, num_devices=B)
    x_d = nc.dram_tensor("x", [H, 2 * CI * W], mmdt, kind="ExternalInput").ap()
    g_d = nc.dram_tensor("g", [H, KH * H], mmdt, kind="ExternalInput").ap()
    w_d = nc.dram_tensor("wt", [128, KW * NSLOT * CO], mmdt,
                         kind="ExternalInput").ap()
    b_d = nc.dram_tensor("bias", [CO, 1], f32, kind="ExternalInput").ap()
    o_d = nc.dram_tensor("out", [CO, H, W], f32, kind="ExternalOutput").ap()

    with tile.TileContext(nc) as tc:
        with (
            tc.tile_pool(name="const", bufs=1) as cpool,
            tc.tile_pool(name="u", bufs=2) as upool,
            tc.tile_pool(name="oacc", bufs=1) as opool,
            tc.tile_pool(name="ps1", bufs=2, space="PSUM") as ps1,
            tc.tile_pool(name="ps2", bufs=2, space="PSUM") as ps2,
        ):
            xT = cpool.tile([H, 2 * CI * W], mmdt)
            nc.sync.dma_start(xT[:], x_d)
            gt = cpool.tile([H, KH * H], mmdt)
            nc.sync.dma_start(gt[:], g_d)
            wt = cpool.tile([128, KW * NSLOT * CO], mmdt)
            nc.sync.dma_start(wt[:], w_d)
            bt = cpool.tile([CO, 1], f32)
            nc.sync.dma_start(bt[:], b_d)

            import concourse.mybir as _mb

            x3 = xT[:].rearrange("p (di w) -> p di w", w=W)   # di = d*64+i

            def stage1(mh, blk):
                u = upool.tile([128, NSLOT * WEXT * MH], mmdt)
                u4 = u[:].rearrange("p (c j m) -> p c j m", c=NSLOT, j=WEXT)
                for j0 in range(0, WEXT, 2):
                    p1 = ps1.tile([128, 1024], f32)
                    for dj in range(2):
                        wg = (blk * WB - HALO + j0 + dj) % W
                        nc.tensor.matmul(p1[:, dj * 512:dj * 512 + KH * MH],
                                         x3[:, :, wg],
                                         gt[:, mh * KH * MH:(mh + 1) * KH * MH],
                                         start=True, stop=True)
                    pv = p1[:].rearrange("p (j s m) -> p j s m", j=2, s=8)
                    # psum s-order [0,2,4,1,3]: half0 cols 0:192, half1 192:320
                    nc.vector.tensor_copy(
                        u4[0:64, :, j0:j0 + 2, :].transpose([0, 2, 1, 3]),
                        pv[0:64, :, 0:3, :])
                    nc.vector.tensor_copy(
                        u4[64:128, 0:2, j0:j0 + 2, :].transpose([0, 2, 1, 3]),
                        pv[64:128, :, 3:5, :])
                return u4

            def stage2(u4, oa3, blk):
                p2 = ps2.tile([128, 2 * 512], f32)
                for t in range(KW):
                    for c in range(NSLOT):
                        kk = 128 if c < 2 else 64
                        lhsT2 = wt[0:kk, (t * NSLOT + c) * CO:
                                   (t * NSLOT + c + 1) * CO]
                        start = (t == 0 and c == 0)
                        stop = (t == KW - 1 and c == NSLOT - 1)
                        for jt in range(WB // JT):
                            # contiguous (j8, m64) = 512 elems
                            rhs2 = u4[0:kk, c,
                                      HALO - t + jt * JT:
                                      HALO - t + (jt + 1) * JT, :]
                            nc.tensor.matmul(
                                p2[:, jt * 512:(jt + 1) * 512], lhsT2, rhs2,
                                start=start, stop=stop)
                p23 = p2[:].rearrange("p (jt j m) -> p jt j m", jt=2, j=JT)
                nc.scalar.activation(
                    oa3[:, :, blk * WB:(blk + 1) * WB]
                        .rearrange("p m (jt j) -> p m jt j", jt=2),
                    p23[:].transpose([0, 3, 1, 2]),
                    _mb.ActivationFunctionType.Identity, bias=bt[:])

            # software pipeline: stage1(k+1) is emitted before stage2(k) so the
            # in-order PE queue fills cast-wait gaps with ready matmul work.
            NBLK = W // WB
            tiles = [(mh, blk) for mh in range(2) for blk in range(NBLK)]
            oaccs = {}
            for mh in range(2):
                oacc = opool.tile([CO, MH * W], f32, tag=f"oacc{mh}")
                oaccs[mh] = oacc[:].rearrange("p (m w) -> p m w", w=W)
            pend = stage1(*tiles[0])
            for k, (mh, blk) in enumerate(tiles):
                nxt = stage1(*tiles[k + 1]) if k + 1 < len(tiles) else None
                stage2(pend, oaccs[mh], blk)
                if blk == NBLK - 1:
                    nc.sync.dma_start(o_d[:, mh * MH:(mh + 1) * MH, :], oaccs[mh])
                pend = nxt
    nc.compile()
    return nc


def _get_prog():
    global _PROG
    if _PROG is None:
        _PROG = _build_program()
    return _PROG


def _build_wstack(weight):
    # wst[(d,i), (t, c, o)]: d=0 -> s=2c ; d=1 -> s=2c+1 (c<2), zeros for c=2
    wst = np.zeros((128, KW * NSLOT * CO), np.float32)
    for t in range(KW):
        for c in range(NSLOT):
            col = (t * NSLOT + c) * CO
            wst[0:64, col:col + CO] = weight[:, :, 2 * c, t].T
            if c < 2:
                wst[64:128, col:col + CO] = weight[:, :, 2 * c + 1, t].T
    return np.ascontiguousarray(wst).astype(_np_dt())


def kernel(x, weight, bias):
    from concourse.bass_utils import run_bass_kernel_spmd

    global _CONSTS
    if _CONSTS is None:
        _CONSTS = _build_consts()
    GT = _CONSTS

    x = np.ascontiguousarray(np.asarray(x, dtype=np.float32))
    weight = np.ascontiguousarray(np.asarray(weight, dtype=np.float32))
    bias = np.ascontiguousarray(np.asarray(bias, dtype=np.float32))

    wst = _build_wstack(weight)
    b2 = np.ascontiguousarray(bias.reshape(CO, 1))

    in_maps = []
    for b in range(B):
        xt = np.ascontiguousarray(x[b].transpose(1, 0, 2)).reshape(H, CI * W)
        xdup = np.ascontiguousarray(
            np.concatenate([xt, xt], axis=1)).astype(_np_dt())
        in_maps.append({"x": xdup, "g": GT, "wt": wst, "bias": b2})

    res = run_bass_kernel_spmd(_get_prog(), in_maps, core_ids=list(range(B)),
                               **_RUN_OPTS)
    global _LAST_RESULT
    _LAST_RESULT = res
    out = np.stack([res.results[b]["out"] for b in range(B)], axis=0)
    return np.ascontiguousarray(out.astype(np.float32))



# revision 9
# speedup vs baseline: 1.1113x; 1.1113x over previous
"""Trainium2 Bass kernel for nn_CCL__69277822485245 (spectral conv via DCT/FFT).

Math: the reference's rFFT along W cancels into a circular 5-tap convolution,
and the DCT-II sandwich M @ diag(D[:,s]) @ D collapses into 5 dense 128x128
matrices G_s (precomputed on host). Per batch element:

    u_s[i, m, w] = sum_h G_s[m, h] x[i, h, w]                  (stage 1)
    out[o, m, n] = sum_{s,t,i} W[o,i,s,t] u_s[i, m, (n-t)%W] + bias[o]   (stage 2)

Sharding: data-parallel over batch B=8 across the 8 NeuronCores (1 each).

v2 layout (per core), designed from the v1 trace (231us):
  - x resident in SBUF as [h=128, (w128, di128)]: stage-1 lhsT = x[:, w, :]
    is contiguous -> fast LDWEIGHTS.  di duplicates i on both partition
    halves (d = s-parity selector).
  - stage 1 runs per m-half (mh): one matmul per w column, N=320 =
    (sidx5, m64) with s-order [0,2,4,1,3]; full W per mh (no halo
    recompute).  psum->SBUF casts are CONTIGUOUS block copies (no element
    transpose): top partitions keep s={0,2,4} (192 el), bottom keep
    s={1,3} (128 el).  Top casts go on the Vector engine, bottom on
    Scalar, window-boundary duplicate casts on GpSimd, spreading the
    evacuation across three engines (v1 had all on Vector w/ transposed
    APs: 163us busy).
  - u lives in per-32-w-block "window" tiles [128, (c3, slot36, m64)]:
    slot j holds w = blk*32 - 4 + j, so stage-2's shifted reads never
    wrap.  The 4 boundary columns are cast twice (into both windows).
  - stage 2 per (blk, jt): 15 accumulation passes (t5 x c3), K=128
    (c<2: s-pair on partition halves; c=2: K=64 s=4 only), rhs =
    contiguous (j8, m64)=512 run of the window tile.  Bias added in the
    single psum->oacc evacuation (Scalar activation, transposing to
    (m, w)).
  - emission interleaves stage-1 matmuls of window k+1 1:1 into stage-2
    matmuls of block k so the in-order PE never waits on casts.
  - x is DMA'd in 5 w-chunks in consumption order; oacc per mh is
    double-buffered and DMA'd while the other mh computes.
"""

import numpy as np

H = 128
W = 128
CI = 64
CO = 128
KH = 5
KW = 5
B = 8

MH = 64          # m-half processed per outer pass
WB = 32          # w-block (stage-2 granularity)
HALO = 4         # extra leading slots per window for the t-shifts
NSLOT = 3        # s-slots per partition half (s = 2c + d)
WIN = WB + HALO  # 36 slots per window tile
NBLK = W // WB   # 4
JT = 8           # j-extent per stage-2 psum tile (N = JT*MH = 512)

_PROG = None
_CONSTS = None
_RUN_OPTS = {}     # test harness may set e.g. {"trace": True, "trace_cores": [0]}
_LAST_RESULT = None


def _np_dt():
    import ml_dtypes
    return ml_dtypes.bfloat16


def _build_consts():
    n = np.arange(H, dtype=np.float64)
    ang = np.pi * (2.0 * n[None, :] + 1.0) * n[:, None] / (2.0 * H)  # [k, h]
    D = 2.0 * np.cos(ang)
    wgt = np.where(n == 0, 0.5, 1.0)
    M = (np.cos(ang).T * wgt[None, :]) / (2.0 * H)                    # [m, k]
    G = np.stack([M @ (D[:, s:s + 1] * D) for s in range(KH)])        # [s, m, h]
    G = G[[0, 2, 4, 1, 3]]   # s-order so each half's psum->u copy is contiguous
    # rhs layout [h, (mh, sidx, ml)]: col = mh*320 + sidx*64 + ml
    GT = (G.transpose(2, 0, 1)                # [h, s, m]
            .reshape(H, KH, 2, MH)            # [h, s, mh, ml]
            .transpose(0, 2, 1, 3)            # [h, mh, s, ml]
            .reshape(H, KH * H))
    return np.ascontiguousarray(GT).astype(_np_dt())


# x DMA chunks (w ranges), in stage-1 consumption order
_XCHUNKS = [(124, 128), (0, 32), (32, 64), (64, 96), (96, 124)]


def _build_program():
    import concourse.mybir as mybir
    import concourse.tile as tile
    from concourse import bacc

    f32 = mybir.dt.float32
    bf16 = mybir.dt.bfloat16

    nc = bacc.Bacc("TRN2", target_bir_lowering=False, debug=False,
                   enable_asserts=False, num_devices=B)
    x_d = nc.dram_tensor("x", [H, W * 2 * CI], bf16, kind="ExternalInput").ap()
    g_d = nc.dram_tensor("g", [H, KH * H], bf16, kind="ExternalInput").ap()
    w_d = nc.dram_tensor("wt", [128, KW * NSLOT * CO], bf16,
                         kind="ExternalInput").ap()
    b_d = nc.dram_tensor("bias", [CO, 1], f32, kind="ExternalInput").ap()
    o_d = nc.dram_tensor("out", [CO, H, W], f32, kind="ExternalOutput").ap()

    with tile.TileContext(nc) as tc:
        with (
            tc.tile_pool(name="const", bufs=1) as cpool,
            tc.tile_pool(name="win", bufs=3) as wpool,
            tc.tile_pool(name="oacc", bufs=1) as opool,
            tc.tile_pool(name="ps1", bufs=4, space="PSUM") as ps1,
            tc.tile_pool(name="ps2", bufs=4, space="PSUM") as ps2,
        ):
            gt = cpool.tile([H, KH * H], bf16)
            nc.sync.dma_start(gt[:], g_d)
            # x in w-chunks so early stage-1 work doesn't wait on all of x
            xts = {}
            for (w0, w1) in _XCHUNKS:
                xt = cpool.tile([H, (w1 - w0) * 2 * CI], bf16,
                                tag=f"x{w0}")
                nc.sync.dma_start(xt[:], x_d[:, w0 * 2 * CI:w1 * 2 * CI])
                xts[(w0, w1)] = xt[:].rearrange("p (w di) -> p w di",
                                                di=2 * CI)
            wt = cpool.tile([128, KW * NSLOT * CO], bf16)
            nc.sync.dma_start(wt[:], w_d)
            bt = cpool.tile([CO, 1], f32)
            nc.sync.dma_start(bt[:], b_d)

            import concourse.mybir as _mb

            def xcol(w):
                for (w0, w1) in _XCHUNKS:
                    if w0 <= w < w1:
                        return xts[(w0, w1)][:, w - w0, :]
                raise AssertionError(w)

            # ---------------- emission units ----------------

            def s1_unit(mh, w, wtile, slot):
                """One stage-1 column: matmul + 2 contiguous psum->SBUF casts."""
                def emit():
                    p1 = ps1.tile([128, KH * MH], f32, name="p1")
                    nc.tensor.matmul(p1[:], xcol(w),
                                     gt[:, mh * KH * MH:(mh + 1) * KH * MH],
                                     start=True, stop=True)
                    w3 = wtile[:].rearrange("p (c j m) -> p c j m",
                                            c=NSLOT, j=WIN)
                    # top half keeps s={0,2,4} -> slots c=0..2 (contiguous 192)
                    nc.vector.tensor_copy(w3[0:64, :, slot, :],
                                          p1[0:64, 0:NSLOT * MH]
                                          .rearrange("p (c m) -> p c m", c=3))
                    # bottom half keeps s={1,3} -> slots c=0..1 (contiguous 128)
                    nc.scalar.copy(w3[64:128, 0:2, slot, :],
                                   p1[64:128, NSLOT * MH:KH * MH]
                                   .rearrange("p (c m) -> p c m", c=2))
                return emit

            def dup_unit(wtile, wtile_next):
                """Seed window k+1 slots 0..3 from window k slots 32..35
                (bf16 SBUF->SBUF on GpSimd; psum is off-limits there)."""
                def emit():
                    w3 = wtile[:].rearrange("p (c j m) -> p c j m",
                                            c=NSLOT, j=WIN)
                    n3 = wtile_next[:].rearrange("p (c j m) -> p c j m",
                                                 c=NSLOT, j=WIN)
                    nc.gpsimd.tensor_copy(n3[:, 0:2, 0:HALO, :],
                                          w3[:, 0:2, WB:WIN, :])
                    nc.gpsimd.tensor_copy(n3[0:64, 2, 0:HALO, :],
                                          w3[0:64, 2, WB:WIN, :])
                return emit

            def s2_mm_unit(wtile, blk, jt, t, c, p2holder):
                def emit():
                    if p2holder[0] is None:
                        p2holder[0] = ps2.tile([128, JT * MH], f32,
                                               name="p2")
                    p2 = p2holder[0]
                    kk = 128 if c < 2 else 64
                    lhsT2 = wt[0:kk, (t * NSLOT + c) * CO:
                               (t * NSLOT + c + 1) * CO]
                    w3 = wtile[:].rearrange("p (c j m) -> p c j m",
                                            c=NSLOT, j=WIN)
                    rhs2 = w3[0:kk, c, jt * JT - t + HALO:
                              (jt + 1) * JT - t + HALO, :]
                    start = (t == 0 and c == 0)
                    stop = (t == KW - 1 and c == NSLOT - 1)
                    nc.tensor.matmul(p2[:], lhsT2, rhs2,
                                     start=start, stop=stop)
                return emit

            def s2_evac_unit(mh, blk, jt, oa, p2holder):
                def emit():
                    p2 = p2holder[0]
                    p23 = p2[:].rearrange("p (j m) -> p j m", j=JT)
                    oa3 = oa[:].rearrange("p (m w) -> p m w", w=W)
                    nc.scalar.activation(
                        oa3[:, :, blk * WB + jt * JT:blk * WB + (jt + 1) * JT],
                        p23[:].transpose([0, 2, 1]),
                        _mb.ActivationFunctionType.Identity, bias=bt[:])
                    p2holder[0] = None
                return emit

            # ---------------- schedule ----------------

            def s1_window_units(mh, k, wtile, wtile_next):
                units = []
                if k == 0:
                    fresh = list(range(124, 128)) + list(range(0, 32))
                else:
                    fresh = list(range(32 * k, 32 * k + 32))
                for w in fresh:
                    if k == 0 and w >= 124:
                        slot = w - 124
                    else:
                        slot = w - (32 * k - 4)
                    units.append(s1_unit(mh, w, wtile, slot))
                if k < NBLK - 1:
                    units.append(dup_unit(wtile, wtile_next))
                return units

            def s2_block_units(mh, k, wtile, oa):
                units = []
                for jt in range(WB // JT):
                    holder = [None]
                    for t in range(KW):
                        for c in range(NSLOT):
                            units.append(s2_mm_unit(wtile, k, jt, t, c,
                                                    holder))
                    units.append(s2_evac_unit(mh, k, jt, oa, holder))
                return units

            # build the full interleaved program
            oaccs = {}
            wtiles = {}

            def get_wtile(mh, k):
                if (mh, k) not in wtiles:
                    wtiles[(mh, k)] = wpool.tile(
                        [128, NSLOT * WIN * MH], bf16, name="win")
                return wtiles[(mh, k)]

            # window list across both mh: (mh, k)
            wins = [(mh, k) for mh in range(2) for k in range(NBLK)]
            for mh in range(2):
                oaccs[mh] = opool.tile([CO, MH * W], f32, tag=f"oacc{mh}",
                                       name=f"oacc{mh}")

            # prologue: first window fully
            mh0, k0 = wins[0]
            t0 = get_wtile(mh0, k0)
            t1 = get_wtile(*wins[1])
            for u in s1_window_units(mh0, k0, t0, t1):
                u()

            for idx, (mh, k) in enumerate(wins):
                wtile = get_wtile(mh, k)
                s2u = s2_block_units(mh, k, wtile, oaccs[mh])
                if idx + 1 < len(wins):
                    nmh, nk = wins[idx + 1]
                    ntile = get_wtile(nmh, nk)
                    nntile = (get_wtile(*wins[idx + 2])
                              if idx + 2 < len(wins) else None)
                    s1u = s1_window_units(nmh, nk, ntile, nntile)
                else:
                    s1u = []
                # interleave 1:1 until s1 exhausted
                for i, u in enumerate(s2u):
                    if i < len(s1u):
                        s1u[i]()
                    u()
                for u in s1u[len(s2u):]:
                    u()
                if k == NBLK - 1:
                    nc.sync.dma_start(
                        o_d[:, mh * MH:(mh + 1) * MH, :],
                        oaccs[mh][:].rearrange("p (m w) -> p m w", w=W))
    nc.compile()
    return nc


def _get_prog():
    global _PROG
    if _PROG is None:
        _PROG = _build_program()
    return _PROG


def _build_wstack(weight):
    # wst[(d,i), (t, c, o)]: d=0 -> s=2c ; d=1 -> s=2c+1 (c<2), zeros for c=2
    wst = np.zeros((128, KW * NSLOT * CO), np.float32)
    for t in range(KW):
        for c in range(NSLOT):
            col = (t * NSLOT + c) * CO
            wst[0:64, col:col + CO] = weight[:, :, 2 * c, t].T
            if c < 2:
                wst[64:128, col:col + CO] = weight[:, :, 2 * c + 1, t].T
    return np.ascontiguousarray(wst).astype(_np_dt())


def kernel(x, weight, bias):
    from concourse.bass_utils import run_bass_kernel_spmd

    global _CONSTS
    if _CONSTS is None:
        _CONSTS = _build_consts()
    GT = _CONSTS

    x = np.ascontiguousarray(np.asarray(x, dtype=np.float32))
    weight = np.ascontiguousarray(np.asarray(weight, dtype=np.float32))
    bias = np.ascontiguousarray(np.asarray(bias, dtype=np.float32))

    wst = _build_wstack(weight)
    b2 = np.ascontiguousarray(bias.reshape(CO, 1))

    in_maps = []
    for b in range(B):
        # [h, (w, di)] with di = d*64 + i duplicated
        xt = x[b].transpose(1, 2, 0)                        # [H, W, ci]
        xdup = np.ascontiguousarray(
            np.concatenate([xt, xt], axis=2).reshape(H, W * 2 * CI)
        ).astype(_np_dt())
        in_maps.append({"x": xdup, "g": GT, "wt": wst, "bias": b2})

    res = run_bass_kernel_spmd(_get_prog(), in_maps, core_ids=list(range(B)),
                               **_RUN_OPTS)
    global _LAST_RESULT
    _LAST_RESULT = res
    out = np.stack([res.results[b]["out"] for b in range(B)], axis=0)
    return np.ascontiguousarray(out.astype(np.float32))
